# revision 5
# baseline (speedup 1.0000x reference)
"""Trainium2 Bass kernel for the Capsule routing module.

Math (per batch b of 32):
    u_hat = inputs[b] @ W            # (N=2048, 128) @ (128, 512) -> (N, 512), e = c*16+d
    b0 = 0
    for i in 0..2:
        cw = softmax(b_i over capsule axis)          # (32, N)
        out = squash(sum_n cw[c,n] * u_hat[n, c-block])   # (32, 16), squash = x/sqrt(sum x^2 + 1e-7)
        b_{i+1}[c,n] = sum_d out[c,d] * u_hat[n, (c,d)]

Key algebraic fusion (u_hat is never materialized):
    raw[c,d]  = sum_n cw[c,n] u_hat[n,(c,d)] = sum_di S_T[di,c] W[di,(c,d)],
                with S_T = inputs^T @ cw^T   (PE, contraction over n)
    b'[n,c]   = sum_di inputs[n,di] V[di,c],
                with V[di,c] = sum_d W[di,(c,d)] out[c,d]  (PE with block-diag rhs)

Sharding: pure data parallel, 4 batches per core across 8 cores. W replicated.
"""

import os
import sys

import numpy as np

if "/opt/trn_rl_repo" not in sys.path:
    sys.path.insert(0, "/opt/trn_rl_repo")

B = 4          # batches per core
N = 2048
D = 128        # d_in
C = 32         # num capsules
DC = 16        # dim capsule
E = C * DC     # 512
P = 128        # partitions
NT = N // P    # 16 n-tiles per batch; n = p*NT + t
NCORES = 8
EPS = 1e-7
ROUTINGS = 3


def _build_program():
    import concourse.bacc as bacc
    import concourse.tile as tile
    from concourse import mybir

    f32 = mybir.dt.float32
    nc = bacc.Bacc("TRN2", debug=False)

    inp_d = nc.dram_tensor("inp", [B, N, D], f32, kind="ExternalInput")
    w_d = nc.dram_tensor("w", [D, E], f32, kind="ExternalInput")
    wt_d = nc.dram_tensor("wt", [P, 4, D], f32, kind="ExternalInput")
    mask_d = nc.dram_tensor("mask", [P, 8], f32, kind="ExternalInput")
    emat_d = nc.dram_tensor("emat", [P, B, B], f32, kind="ExternalInput")
    ident_d = nc.dram_tensor("ident", [P, P], f32, kind="ExternalInput")
    out_d = nc.dram_tensor("out", [B, E], f32, kind="ExternalOutput")

    mult = mybir.AluOpType.mult
    AF = mybir.ActivationFunctionType
    X = mybir.AxisListType.X

    from contextlib import ExitStack

    with tile.TileContext(nc) as tc, ExitStack() as ctx:
        consts = ctx.enter_context(tc.tile_pool(name="consts", bufs=1))
        big = ctx.enter_context(tc.tile_pool(name="big", bufs=1))
        work = ctx.enter_context(tc.tile_pool(name="work", bufs=2))
        ps_tr = ctx.enter_context(tc.tile_pool(name="ps_tr", bufs=2, space="PSUM"))
        ps_s = ctx.enter_context(tc.tile_pool(name="ps_s", bufs=1, space="PSUM"))
        ps_raw = ctx.enter_context(tc.tile_pool(name="ps_raw", bufs=1, space="PSUM"))
        ps_small = ctx.enter_context(tc.tile_pool(name="ps_small", bufs=1, space="PSUM"))
        ps_bt = ctx.enter_context(tc.tile_pool(name="ps_bt", bufs=2, space="PSUM"))

        # ---- constants ----
        w_sb = consts.tile([P, C, DC], f32, tag="w")
        nc.sync.dma_start(out=w_sb, in_=w_d.ap().rearrange("p (c d) -> p c d", d=DC))
        wt_sb = consts.tile([P, 4, D], f32, tag="wt")
        nc.sync.dma_start(out=wt_sb, in_=wt_d.ap())
        mask_sb = consts.tile([P, 8], f32, tag="mask")
        nc.sync.dma_start(out=mask_sb, in_=mask_d.ap())
        emat_sb = consts.tile([P, B, B], f32, tag="emat")
        nc.sync.dma_start(out=emat_sb, in_=emat_d.ap())
        ident_sb = consts.tile([P, P], f32, tag="ident")
        nc.sync.dma_start(out=ident_sb, in_=ident_d.ap())
        uni_sb = consts.tile([P, C], f32, tag="uni")
        nc.vector.memset(uni_sb, 1.0 / C)
        eps_sb = consts.tile([B, 1], f32, tag="eps")
        nc.vector.memset(eps_sb, EPS)

        # ---- persistent buffers ----
        nat = [big.tile([P, NT, D], f32, tag=f"nat{b}", name=f"nat{b}") for b in range(B)]
        natT = [big.tile([P, NT, P], f32, tag=f"natT{b}", name=f"natT{b}") for b in range(B)]
        S_all = big.tile([P, B, C], f32, tag="S_all")
        exp_all = big.tile([P, B, NT, C], f32, tag="exp_all")
        rz_all = big.tile([P, B, NT], f32, tag="rz_all")
        cw_all = big.tile([P, B, NT, C], f32, tag="cw_all")
        V_all = big.tile([P, B, C], f32, tag="V_all")
        sq = big.tile([B, C, DC], f32, tag="sq")
        ss = big.tile([B, C], f32, tag="ss")
        srt = big.tile([B, C], f32, tag="srt")
        rs = big.tile([B, C], f32, tag="rs")
        outf = big.tile([B, C, DC], f32, tag="outf")
        outc = big.tile([P, 4, B], f32, tag="outc")
        BD = big.tile([P, 4, B, 8], f32, tag="BD")
        ztmp = big.tile([P, B, NT], f32, tag="ztmp")

        # ---- load inputs (contiguous per partition: n = p*NT + t) ----
        for b in range(B):
            nc.sync.dma_start(
                out=nat[b].rearrange("p t d -> p (t d)"),
                in_=inp_d.ap()[b].rearrange("(p t) d -> p (t d)", t=NT),
            )

        # ---- transpose inputs: natT[d, t, p] = nat[p, t, d] ----
        for b in range(B):
            for q in range(4):
                tr = ps_tr.tile([P, 4, P], f32, tag="tr")
                for j in range(4):
                    t = q * 4 + j
                    nc.tensor.transpose(tr[:, j, :], nat[b][:, t, :], ident_sb)
                nc.scalar.copy(natT[b][:, q * 4:(q + 1) * 4, :], tr)

        # ---- routing iterations ----
        for it in range(ROUTINGS):
            # S_T[di, b, c] = sum_n inputs[b, n, di] * cw[b, n, c]
            S_ps = ps_s.tile([P, B, C], f32, tag="S_ps")
            for b in range(B):
                for t in range(NT):
                    rhs = uni_sb if it == 0 else cw_all[:, b, t, :]
                    nc.tensor.matmul(
                        S_ps[:, b, :], nat[b][:, t, :], rhs,
                        start=(t == 0), stop=(t == NT - 1),
                    )
            nc.scalar.copy(S_all, S_ps)

            # raw[b, e] = sum_di S_T[di, b, c(e)] * W[di, e]   (pre-squash outputs)
            raw_ps = ps_raw.tile([B, C, DC], f32, tag="raw_ps")
            for b in range(B):
                Pb = work.tile([P, C, DC], f32, tag="Pb")
                eng = nc.vector if b % 2 == 0 else nc.gpsimd
                eng.tensor_mul(
                    Pb, w_sb,
                    S_all[:, b, :, None].broadcast_to([P, C, DC]),
                )
                nc.tensor.matmul(
                    raw_ps.rearrange("b c d -> b (c d)"),
                    emat_sb[:, b, :],
                    Pb.rearrange("p c d -> p (c d)"),
                    start=(b == 0), stop=(b == B - 1),
                )

            # squash: outf = raw / sqrt(sum_d raw^2 + eps)
            nc.scalar.activation(sq, raw_ps, AF.Square)
            nc.vector.tensor_reduce(ss[:, :, None], sq, axis=X, op=mybir.AluOpType.add)
            nc.scalar.activation(srt, ss, AF.Sqrt, bias=eps_sb)
            nc.vector.reciprocal(rs, srt)
            nc.vector.tensor_mul(
                outf, raw_ps, rs[:, :, None].broadcast_to([B, C, DC])
            )

            if it == ROUTINGS - 1:
                nc.sync.dma_start(
                    out=out_d.ap(), in_=outf.rearrange("b c d -> b (c d)")
                )
                break

            # outc[p, q, b] = outf[b, q*128 + p]  (transpose rows -> columns)
            oc_ps = ps_small.tile([P, 4, B], f32, tag="oc_ps")
            outf_flat = outf.rearrange("b c d -> b (c d)")
            for q in range(4):
                nc.tensor.transpose(
                    oc_ps[:, q, :], outf_flat[:, q * P:(q + 1) * P],
                    ident_sb[0:B, 0:B],
                )
            nc.scalar.copy(outc, oc_ps)

            # BD[p, q, b, j] = outc[p, q, b] * mask[p, j]
            nc.vector.tensor_mul(
                BD,
                outc[:, :, :, None].broadcast_to([P, 4, B, 8]),
                mask_sb[:, None, None, :].broadcast_to([P, 4, B, 8]),
            )

            # V[di, b, c] = sum_d W[di, (c,d)] * outf[b, (c,d)]
            V_ps = ps_small.tile([P, B, C], f32, tag="V_ps")
            for q in range(4):
                for b in range(B):
                    nc.tensor.matmul(
                        V_ps[:, b, q * 8:(q + 1) * 8],
                        wt_sb[:, q, :],
                        BD[:, q, b, :],
                        start=True, stop=True,
                    )
            nc.scalar.copy(V_all, V_ps)

            # b-pass + softmax per batch
            for b in range(B):
                bT = ps_bt.tile([P, NT, C], f32, tag="bT")
                for t in range(NT):
                    nc.tensor.matmul(
                        bT[:, t, :], natT[b][:, t, :], V_all[:, b, :],
                        start=True, stop=True,
                    )
                nc.scalar.activation(exp_all[:, b], bT, AF.Exp)
                nc.vector.tensor_reduce(
                    ztmp[:, b, :, None], exp_all[:, b], axis=X,
                    op=mybir.AluOpType.add,
                )
                nc.vector.reciprocal(rz_all[:, b], ztmp[:, b])
                eng = nc.vector if b % 2 == 0 else nc.gpsimd
                eng.tensor_mul(
                    cw_all[:, b], exp_all[:, b],
                    rz_all[:, b][:, :, None].broadcast_to([P, NT, C]),
                )

    nc.compile()
    return nc


_PROGRAM = None


def _get_program():
    global _PROGRAM
    if _PROGRAM is None:
        _PROGRAM = _build_program()
    return _PROGRAM


def _host_consts(W2d):
    wt = np.ascontiguousarray(
        W2d.T.reshape(4, P, D).transpose(1, 0, 2)
    )  # wt[p, q, di] = W[di, q*128+p]
    mask = np.zeros((P, 8), dtype=np.float32)
    for j in range(8):
        mask[j * 16:(j + 1) * 16, j] = 1.0
    emat = np.zeros((P, B, B), dtype=np.float32)
    for b in range(B):
        emat[:, b, b] = 1.0
    ident = np.eye(P, dtype=np.float32)
    return wt, mask, emat, ident


def kernel(inputs: np.ndarray, W: np.ndarray) -> np.ndarray:
    from concourse.bass_utils import run_bass_kernel_spmd

    nc = _get_program()
    inputs = np.ascontiguousarray(inputs, dtype=np.float32)
    W2d = np.ascontiguousarray(W.reshape(D, E), dtype=np.float32)
    wt, mask, emat, ident = _host_consts(W2d)

    in_maps = []
    for i in range(NCORES):
        in_maps.append({
            "inp": inputs[i * B:(i + 1) * B],
            "w": W2d,
            "wt": wt,
            "mask": mask,
            "emat": emat,
            "ident": ident,
        })
    res = run_bass_kernel_spmd(nc, in_maps, list(range(NCORES)))
    out = np.concatenate(
        [res.results[i]["out"].reshape(B, C, DC) for i in range(NCORES)], axis=0
    )
    return out
